# revision 29
# baseline (speedup 1.0000x reference)
"""Trainium2 Bass kernel for nn_DistanceLayer (gaussian-prior distance attention).

Math: out[b,i] = sum_j softmax_j(q_i.k_j * MD^-0.5 * prior(j-i))[j] * (j-i)

The gaussian prior (std=1) underflows so fast in f32 that outside a small
band |j-i| <= 7 the f32 score is exactly 0, so exp(score) is exactly 1.0.
Each softmax row is a small band of "interesting" values plus a uniform
far field with closed-form sums:

    T0_i = (N - win) + sum_window exp(s)            (denominator)
    T1_i = C1_i + sum_window exp(s)*jrel + ws_i * sum_window exp(s)
    out_i = T1_i / T0_i - i

where C1_i = sum_all_j j - sum_window_i j (exact ints in f32) and ws_i is
the window start of row i's 32-row quarter.  In-window far entries have
score exactly 0 (prior pattern is 0 outside the band) and contribute
exp(0)=1, which the constants account for.

v4 layout/schedule:
  - 32-row QUARTER windows (win=48 vs 80 for 64-row halves): 40% fewer
    score elements, so the DVE-locked postprocessing (prior-mul and the
    window reduces, which have no 2x dtype mode) fits the engine budget.
    PE pays ~64 small band matmuls, but has DoubleRow headroom.
  - fp8e4m3 x/weights with DoubleRow matmuls: one PE instruction per
    (chunk, projection) at 0.5 cycles/row.
  - q and k of a chunk share one [P, 2, 512] PSUM pair and evict in a
    single fused op (ACT-heavy split).
  - band groups of (3,4,4,3,2) tiles: group 0 only needs k chunk 0 so
    postprocessing starts at the first eviction, and the last group's
    serial tail is short.  exp output and e*jrel share one tile; ONE
    reduce per group yields both sums interleaved (col 2t / 2t+1).
  - input DMAs ordered for earliest projection unlock: sync (xtA, xtC,
    patterns), scalar (w2, xtB, xtD), gpsimd (combine consts); PE junk
    matmuls ramp the 0.65->2.4GHz clock until real data lands.

Sharding: pure data-parallel over batch B=8 across the 8 cores.
"""

import sys

sys.path.insert(0, "/opt/trn_rl_repo")

import ml_dtypes
import numpy as np

import concourse.bacc as bacc
import concourse.tile as tile
from concourse import mybir
from concourse.bass_utils import run_bass_kernel_spmd

B, N, D, MD = 8, 2048, 256, 128
NCORES = 8
P = 128
QR = 32  # quarter height
NQ = N // QR  # 64 quarters
NT = N // P  # 16 row tiles
GROUPS = (3, 4, 4, 3, 2)  # band group sizes in tiles
GSTART = (0, 3, 7, 11, 14)
# group g's k windows reach into proj chunk UNLOCK[g]; emitted after it
UNLOCK = (0, 1, 2, 3, 3)
DCH = D // P  # 2 contraction chunks
PROJ_CHUNK = 512
NPC = N // PROJ_CHUNK  # 4 projection column chunks
PI = 3.1415926  # matches reference
WSCALE = 8.0  # fp8 weight pre-scale; pattern divides the x64 back out
F32 = mybir.dt.float32
BF16 = mybir.dt.bfloat16
F8 = mybir.dt.float8e4

_cache = {}
# exposed for test harness profiling: (nc, in_maps)
last_run = None


def _plan_band(prior_mean, prior_std):
    """f32 prior over every offset, exactly as the reference computes it,
    and the band of offsets whose scores can round exp() away from 1.0."""
    d = np.arange(-(N - 1), N, dtype=np.float32)
    ps = np.float32(prior_std)
    pm = np.float32(prior_mean)
    prior = (
        np.float32(1.0)
        / ps
        / np.sqrt(np.float32(2.0) * np.float32(PI))
        * np.exp(np.float32(-0.5) * (d - pm) ** 2 / ps**2)
    ).astype(np.float32)
    # |score| <= |prior| * |q.k*scale| ; bound the latter by 1024 (actual
    # max is ~7 for these glorot inputs).  exp(x) rounds to 1.0f for
    # |x| < 2^-26; use 2^-27 for margin.
    sig = np.abs(prior) * 1024.0 >= 2.0**-27
    if not sig.any():
        dlo, dhi = 0, 0
    else:
        dlo = int(d[sig].min())
        dhi = int(d[sig].max())
    return prior, dlo, dhi


def _window_geometry(dlo, dhi):
    """Per-quarter window starts ws4[64] plus deduplicated per-group
    prior patterns.  Pattern key for a group is (gt, rel offsets...) of
    its quarter-windows relative to the group's base row."""
    span = dhi - dlo
    win = QR + span + 1
    win = max(48, ((win + 15) // 16) * 16)
    assert win <= 192, f"prior band too wide for quarter-banded kernel: {dlo}..{dhi}"
    extra = win - (QR + span)
    ws4 = []
    for h in range(NQ):
        ws = min(max(h * QR + dlo - extra // 2, 0), N - win)
        lo_need = max(0, h * QR + dlo)
        hi_need = min(N - 1, h * QR + QR - 1 + dhi)
        assert ws <= lo_need and hi_need < ws + win, (h, ws, lo_need, hi_need)
        ws4.append(ws)
    gkeys = []
    for g, gt in enumerate(GROUPS):
        t0 = GSTART[g]
        base = t0 * P
        gkeys.append((gt,) + tuple(ws4[4 * t0 + i] - base for i in range(4 * gt)))
    key_vals = sorted(set(gkeys))
    key_idx = [key_vals.index(k) for k in gkeys]
    key_off = {}
    off = 0
    for k in key_vals:
        key_off[k] = off
        off += k[0] * win
    return win, ws4, key_vals, key_idx, key_off, off


def _build(win, ws4, key_idx, key_off_list, pat_cols, use_bias):
    nc = bacc.Bacc()
    GWMAX = max(GROUPS) * win

    # f32 consts: c1 | wsm | ii | bq8 | bk8
    O_C1, O_WS, O_II, O_BQ = 0, NT, 2 * NT, 3 * NT
    CW = 3 * NT + 2
    # bf16 consts: patterns then j0
    PJW = pat_cols + GWMAX
    O_J0 = pat_cols

    xt_d = nc.dram_tensor("xt", [P, DCH * N], F8, kind="ExternalInput")
    w2_d = nc.dram_tensor("w2", [P, 2 * DCH * MD], F8, kind="ExternalInput")
    cs_d = nc.dram_tensor("cst", [P, CW], F32, kind="ExternalInput")
    pj_d = nc.dram_tensor("pj", [P, PJW], BF16, kind="ExternalInput")
    y_d = nc.dram_tensor("y", [P, NT], F32, kind="ExternalOutput")

    with tile.TileContext(nc) as tc:
        with (
            tc.tile_pool(name="const", bufs=1) as const,
            tc.tile_pool(name="psum_proj", bufs=3, space="PSUM") as psum_proj,
            tc.tile_pool(name="psum_band", bufs=2, space="PSUM") as psum_band,
            tc.tile_pool(name="band_sp", bufs=2) as sp_pool,
            tc.tile_pool(name="band_ee", bufs=2) as ee_pool,
            tc.tile_pool(name="comb", bufs=1) as comb,
        ):
            # ---- engine warmups (emitted first so DVE/PE start at body
            # entry) ----
            # PE: junk matmuls flip the HAM clock gate (full speed needs
            # ~3us of continuous busy) while the input DMAs land.  ACT:
            # one tiny Exp pulls the 1.3us ACT_TABLE_LOAD off the
            # critical path.
            wtile = const.tile([P, GWMAX], BF16, tag="warm_w")
            nc.vector.memset(wtile, 0.0)
            wact_in = const.tile([P, 1], F32, tag="warm_a")
            nc.vector.memset(wact_in, 0.0)
            wact_out = const.tile([P, 1], F32, tag="warm_ao")
            nc.scalar.activation(
                out=wact_out, in_=wact_in, func=mybir.ActivationFunctionType.Exp
            )

            # ---- input DMAs: sync carries w2 + the c0 half of x, gpsimd
            # carries c1 + patterns + consts; ACT issues none so it is
            # free for evictions/exp.  Two big (2KB/partition) x
            # transfers beat four small ones: DMA here is
            # descriptor-latency-bound, not byte-bound. ----
            w2_s = const.tile([P, 2 * DCH * MD], F8, tag="w2")
            xt_s = const.tile([P, DCH * N], F8, tag="xt")
            pj_s = const.tile([P, PJW], BF16, tag="pj")
            cs_s = const.tile([P, CW], F32, tag="cst")
            # chunk 0's x slices ship first and small so the first
            # projection unlocks as early as possible (per-transfer
            # completion sems + ~0.9us propagation make granularity
            # matter); w2 is tiny and rides second on sync
            PC = PROJ_CHUNK
            for c in range(DCH):  # sync carries c0, gpsimd carries c1
                eng = nc.sync if c == 0 else nc.gpsimd
                for lo, hi in ((0, PC), (PC, 2 * PC), (2 * PC, N)):
                    eng.dma_start(
                        out=xt_s[:, c * N + lo : c * N + hi],
                        in_=xt_d[:, c * N + lo : c * N + hi],
                    )
                    if c == 0 and hi == PC:
                        nc.sync.dma_start(out=w2_s, in_=w2_d[:, :])
            nc.gpsimd.dma_start(out=pj_s, in_=pj_d[:, :])
            nc.gpsimd.dma_start(out=cs_s, in_=cs_d[:, :])

            for _ in range(11):
                wps = psum_band.tile([P, GWMAX], F32, tag="band")
                nc.tensor.matmul(
                    wps, lhsT=wtile[:, :P], rhs=wtile, start=True, stop=True
                )

            qkT = const.tile([P, 2 * N], BF16, tag="qkT")  # q | k
            # interleaved sums: col 2t = sum_e[t], 2t+1 = sum_ec[t]
            sums = const.tile([P, 2 * NT], BF16, tag="sums")

            # ---- projection chunk: q and k into one [P, 1024] psum pair,
            # each a single fp8 DoubleRow matmul (contraction pairs are the
            # two D-halves).  Early chunks evict split (ACT does k, DVE
            # does q, in parallel) to unlock the first band groups sooner;
            # later chunks evict fused on ACT. ----
            EVICT_ENG = ["split", "split", "act", "act"]

            def emit_proj(n4):
                ps_t = psum_proj.tile([P, 2 * PROJ_CHUNK], F32, tag="proj")
                rhs3 = xt_s[:].rearrange("p (c j) -> p c j", c=DCH)[
                    :, :, n4 * PROJ_CHUNK : (n4 + 1) * PROJ_CHUNK
                ]
                for pj in range(2):  # 0=q, 1=k
                    lhsT3 = w2_s[
                        :, 2 * pj * MD : (2 * pj + 2) * MD
                    ].rearrange("p (c m) -> p c m", c=DCH)
                    nc.tensor.matmul(
                        ps_t[:, pj * PROJ_CHUNK : (pj + 1) * PROJ_CHUNK],
                        lhsT=lhsT3,
                        rhs=rhs3,
                        start=True,
                        stop=True,
                        perf_mode=mybir.MatmulPerfMode.DoubleRow,
                    )
                # fused eviction: [P, 2, 512] view of qkT at (q, k) slices
                dst = qkT[:].rearrange("p (s j) -> p s j", s=2)[
                    :, :, n4 * PROJ_CHUNK : (n4 + 1) * PROJ_CHUNK
                ]
                src = ps_t[:].rearrange("p (s j) -> p s j", s=2)
                eng = EVICT_ENG[n4]
                if use_bias:
                    # per-partition bias differs for q and k: two ops
                    for pj in range(2):
                        b_s = cs_s[:, O_BQ + pj : O_BQ + pj + 1]
                        d1 = qkT[:, pj * N + n4 * PROJ_CHUNK : pj * N + (n4 + 1) * PROJ_CHUNK]
                        s1 = ps_t[:, pj * PROJ_CHUNK : (pj + 1) * PROJ_CHUNK]
                        if eng == "act":
                            nc.scalar.activation(
                                out=d1, in_=s1,
                                func=mybir.ActivationFunctionType.Identity,
                                bias=b_s, scale=1.0,
                            )
                        else:
                            nc.vector.tensor_scalar_add(d1, s1, b_s)
                else:
                    if eng == "act":
                        nc.scalar.copy(out=dst, in_=src)
                    elif eng == "dve":
                        nc.vector.tensor_copy(dst, src)
                    else:  # split: ACT takes k, DVE takes q, in parallel
                        nc.scalar.copy(
                            out=qkT[:, N + n4 * PROJ_CHUNK : N + (n4 + 1) * PROJ_CHUNK],
                            in_=ps_t[:, PROJ_CHUNK:],
                        )
                        nc.vector.tensor_copy(
                            qkT[:, n4 * PROJ_CHUNK : (n4 + 1) * PROJ_CHUNK],
                            ps_t[:, :PROJ_CHUNK],
                        )

            # ---- band group: 4*gt quarter matmuls, postproc in one pass ----
            def emit_group(g, defer_reduce=False):
                t0, gt = GSTART[g], GROUPS[g]
                gw = gt * win
                ps_full = psum_band.tile([P, GWMAX], F32, tag="band")
                ps_s = ps_full[:, :gw]
                for tb in range(gt):
                    t = t0 + tb
                    for qd in range(4):
                        ws = ws4[4 * t + qd]
                        nc.tensor.matmul(
                            ps_s[qd * QR : (qd + 1) * QR, tb * win : (tb + 1) * win],
                            lhsT=qkT[:, t * P + qd * QR : t * P + (qd + 1) * QR],
                            rhs=qkT[:, N + ws : N + ws + win],
                            start=True,
                            stop=True,
                            tile_position=(0, qd * QR),
                        )
                pat = pj_s[:, key_off_list[g] : key_off_list[g] + gw]
                sp_full = sp_pool.tile([P, GWMAX], BF16, tag="sp")
                sp_t = sp_full[:, :gw]
                nc.vector.tensor_mul(sp_t, ps_s, pat)
                ee_full = ee_pool.tile([P, 2 * GWMAX], BF16, tag="ee")
                ee_t = ee_full[:, : 2 * gw]
                nc.scalar.activation(
                    out=ee_t[:, :gw], in_=sp_t,
                    func=mybir.ActivationFunctionType.Exp,
                )
                # last group's ej on DVE: shorter tail chain (no Pool hop)
                ej_eng = nc.vector if g == len(GROUPS) - 1 else nc.gpsimd
                ej_eng.tensor_mul(
                    ee_t[:, gw : 2 * gw], ee_t[:, :gw], pj_s[:, O_J0 : O_J0 + gw]
                )
                # one reduce for both sums; out cols interleave as
                # (kind, tile) -> 2*(t0+tb)+kind via a [2, gt] out view
                out_ap = sums[:, 2 * t0 : 2 * (t0 + gt)].rearrange(
                    "p (t k) -> p k t", k=2
                )
                # bf16 sums: worst-case 0.4% of ~8e3 is ~0.02 abs on a
                # +-1023-scale output (tolerance 2e-2 rel) — safe.
                def do_reduce():
                    with nc.allow_low_precision("bf16 window sums, ~1e-5 rel out err"):
                        nc.vector.tensor_reduce(
                            out=out_ap,
                            in_=ee_t.rearrange("p (t w) -> p t w", w=win),
                            axis=mybir.AxisListType.X,
                            op=mybir.AluOpType.add,
                        )

                if defer_reduce:
                    return do_reduce
                do_reduce()

            # ---- combine: out = (c1 + sum_ec + ws*sum_e)/(N-win+sum_e) - i ----
            c1_s = cs_s[:, O_C1 : O_C1 + NT]
            ws_s = cs_s[:, O_WS : O_WS + NT]
            ii_s = cs_s[:, O_II : O_II + NT]
            outv2 = comb.tile([P, NT], F32, tag="outv2")

            def emit_combine(sl, spine_v=False):
                # numerator builds on Pool; the t0->rec->outv->outv2 spine
                # runs on Pool+DVE for the big early pass (keeps DVE free
                # for reduces) and entirely on DVE for the short final
                # pass (no cross-engine hops on the critical tail)
                w = sl.stop - sl.start
                se = sums[:, 2 * sl.start : 2 * sl.stop].rearrange(
                    "p (t k) -> p t k", k=2
                )[:, :, 0]
                sec = sums[:, 2 * sl.start : 2 * sl.stop].rearrange(
                    "p (t k) -> p t k", k=2
                )[:, :, 1]
                spine = nc.vector if spine_v else nc.gpsimd
                tmp = comb.tile([P, w], F32, tag="tmp")
                nc.gpsimd.tensor_mul(tmp, ws_s[:, sl], se)
                num = comb.tile([P, w], F32, tag="num")
                nc.gpsimd.tensor_add(num, c1_s[:, sl], sec)
                num2 = comb.tile([P, w], F32, tag="num2")
                nc.gpsimd.tensor_add(num2, num, tmp)
                t0 = comb.tile([P, w], F32, tag="t0")
                spine.tensor_scalar_add(t0, se, float(N - win))
                rec = comb.tile([P, w], F32, tag="rec")
                nc.vector.reciprocal(rec, t0)
                outv = comb.tile([P, w], F32, tag="outv")
                spine.tensor_mul(outv, num2, rec)
                spine.tensor_sub(outv2[:, sl], outv, ii_s[:, sl])
                nc.sync.dma_start(out=y_d[:, sl], in_=outv2[:, sl])

            # shift-by-one: proj n+1's eviction is emitted before group
            # n-1's postprocessing so the engine FIFOs never make a band
            # group wait behind postproc of an earlier group
            emit_proj(0)
            emit_proj(1)
            emit_group(0)  # tiles 0-2, k cols < 512
            emit_group(1)  # tiles 3-6, k < 1024
            emit_proj(2)
            emit_group(2)  # tiles 7-10, k < 1536
            emit_proj(3)
            emit_group(3)  # tiles 11-13
            red4 = emit_group(4, defer_reduce=True)  # tiles 14-15
            emit_combine(slice(0, 14))  # runs under g4's exp/ej
            red4()
            emit_combine(slice(14, NT), spine_v=True)  # short final tail

    nc.finalize()
    return nc


def kernel(x, Wq, bq, Wk, bk, prior_mean, prior_std):
    global last_run
    x = np.asarray(x, dtype=np.float32)
    Wq = np.asarray(Wq, dtype=np.float32)
    Wk = np.asarray(Wk, dtype=np.float32)
    bq = np.asarray(bq, dtype=np.float32)
    bk = np.asarray(bk, dtype=np.float32)

    prior, dlo, dhi = _plan_band(
        float(np.asarray(prior_mean)[0]), float(np.asarray(prior_std)[0])
    )
    win, ws4, key_vals, key_idx, key_off, pat_cols = _window_geometry(dlo, dhi)
    use_bias = bool(np.any(bq != 0.0) or np.any(bk != 0.0))
    key_off_list = [key_off[key_vals[key_idx[g]]] for g in range(len(GROUPS))]

    ckey = (win, tuple(ws4), tuple(key_idx), use_bias)
    if ckey not in _cache:
        _cache[ckey] = _build(win, ws4, key_idx, key_off_list, pat_cols, use_bias)
    nc = _cache[ckey]

    bf = ml_dtypes.bfloat16
    f8 = ml_dtypes.float8_e4m3
    scale = np.float32(MD**-0.5) / np.float32(WSCALE * WSCALE)
    GWMAX = max(GROUPS) * win

    # prior*scale patterns per distinct key, then j0
    p_idx = np.arange(P)[:, None]
    c_idx = np.arange(win)[None, :]
    quad = np.arange(P) // QR  # quarter index of each partition
    pj = np.zeros((P, pat_cols + GWMAX), np.float32)
    for kv in key_vals:
        gt, rel = kv[0], kv[1:]
        off = key_off[kv]
        for tb in range(gt):
            relcol = np.asarray(rel)[4 * tb + quad][:, None]
            dm = c_idx + relcol - P * tb - p_idx
            pj[:, off + tb * win : off + (tb + 1) * win] = np.where(
                (dm >= dlo) & (dm <= dhi), prior[dm + N - 1] * scale, np.float32(0.0)
            )
    pj[:, pat_cols:] = np.tile(np.arange(win, dtype=np.float32), max(GROUPS))[None, :]

    sumj_all = float(N * (N - 1) // 2)
    c1 = np.zeros((P, NT), np.float32)
    wsm = np.zeros((P, NT), np.float32)
    ii = np.zeros((P, NT), np.float32)
    ws4a = np.asarray(ws4, np.float32)
    for t in range(NT):
        wsv = ws4a[4 * t + quad]
        c1[:, t] = sumj_all - (win * wsv + win * (win - 1) // 2)
        wsm[:, t] = wsv
        ii[:, t] = t * P + np.arange(P)

    # f32 consts: c1 | wsm | ii | 8*bq | 8*bk  (weights ship pre-scaled x8,
    # so the bias folded into the eviction must match)
    cst = np.ascontiguousarray(
        np.concatenate(
            [
                c1,
                wsm,
                ii,
                np.float32(WSCALE) * bq.reshape(P, 1),
                np.float32(WSCALE) * bk.reshape(P, 1),
            ],
            axis=1,
        ).astype(np.float32)
    )
    pj16 = np.ascontiguousarray(pj.astype(bf))

    # weights: wq chunks then wk chunks, [P, 4*MD], fp8 at x8 scale
    wq_h = (Wq * WSCALE).reshape(DCH, P, MD).transpose(1, 0, 2).reshape(P, DCH * MD)
    wk_h = (Wk * WSCALE).reshape(DCH, P, MD).transpose(1, 0, 2).reshape(P, DCH * MD)
    w2_h = np.ascontiguousarray(np.concatenate([wq_h, wk_h], axis=1)).astype(f8)

    in_maps = []
    for core in range(NCORES):
        xb = x[core]  # [N, D]
        # xt[p, c*N + j] = x[j, c*128 + p]
        xt_h = np.ascontiguousarray(
            xb.T.reshape(DCH, P, N).transpose(1, 0, 2).reshape(P, DCH * N)
        ).astype(f8)
        in_maps.append({"xt": xt_h, "w2": w2_h, "cst": cst, "pj": pj16})

    res = run_bass_kernel_spmd(nc, in_maps, list(range(NCORES)))
    last_run = (nc, in_maps)
    # y[p, t] = out[128t + p]  ->  out = y.T.flatten()
    out = np.stack(
        [res.results[c]["y"].T.reshape(-1) for c in range(NCORES)], axis=0
    )
    return out.astype(np.float32)
